# revision 5
# baseline (speedup 1.0000x reference)
"""Trainium2 Bass kernel for nn_MultiHeadAttnC (QANet-style self-attention).

Reference computation (per batch b):
    memory = w_mem @ queries[b]          # [2D, L]  (pointwise conv)
    query  = w_query @ queries[b]        # [D, L]
    K, V   = heads of memory             # H=8 heads, DH=16
    Q      = heads of query * DH^-0.5
    S      = Q @ K^T  (masked over kv)   # [H, L, L]
    out[b] = softmax(S) @ V  -> recombined to [D, L]

Strategy (v5):
  - Data parallel: batch b -> NeuronCore b. Weights replicated. No collectives.
  - The exp (27M/core, PSUM-resident so only ACT+DVE can touch it) is the
    roofline: ~117us at the two engines' combined ~1.8 cols/ns. Everything
    else is arranged so ACT/DVE do almost nothing but exp:
      - ONE PSUM accumulator bank per (jq, X) stream: both j-streams'
        AV quads write disjoint partition ranges of the same bank
        (emit 4 col-tiled matmuls), so a stream drains with a single
        [128, 512] copy. accp bufs=2 double-buffers across streams.
      - Drains and epilogues are EMITTED one stream late, so on each
        engine's in-order queue they sit behind ~13 chunks of exps and
        never stall waiting for the AV tail.
      - Epilogue: per (jq, X) one strided pk DMA (denominator rows), one
        broadcast rb DMA (band layout), final multiply on GpSimd (SBUF
        only), one scatter DMA straight to DRAM (head-major remap in the
        DMA AP). No xt staging, no engine work besides 1 reciprocal.
      - proj-V evacuation batches 4 chunks per PSUM bank -> 1 strided
        copy per 4 chunks.
  - K-major attention, heads split 4+4 over two weight planes; S^T duo
    tiles are [128 kv, 2x512 q] (2 banks), quad row-banded matmuls merge
    on the PE (~216ns/group measured). Masked kv compacted host-side.
  - exp split between ACT (native Exp) and DVE (single-op Schraudolph:
    i16 = round(x*A + B) == bf16 bits of exp(x)) by a greedy time ledger
    with HW-calibrated costs.
"""

import numpy as np
from contextlib import ExitStack

import concourse.bass as bass
import concourse.tile as tile
from concourse import bacc, mybir
from concourse import bass_utils

B, D, L, H, DH = 8, 128, 2048, 8, 16
f32 = mybir.dt.float32
bf16 = mybir.dt.bfloat16
i16 = mybir.dt.int16
f32r = mybir.dt.float32r
IN_DT = f32r
QT = 512             # q columns per stream tile
NJQ = L // QT        # 4
NXP = 2              # weight spread planes (4 head-groups each)

# Schraudolph exp constants for round-to-nearest f32->i16 conversion:
# bf16_bits(exp(x)) ~= round(x * 2^7*log2(e) + (127*2^7 - 5.6))
EXP_A = 184.6649652337873
EXP_B = 16250.4

_program_cache: dict = {}


def _body(ctx, tc, qf_d, qkv_d, wq_d, wk_d, wv_d, val_d, out_d, n_kv, compact):
    nc = tc.nc
    Lkv = n_kv * 128
    Exp = mybir.ActivationFunctionType.Exp
    Copy = mybir.ActivationFunctionType.Copy
    mult, add = mybir.AluOpType.mult, mybir.AluOpType.add
    NX = NXP

    consts = ctx.enter_context(tc.tile_pool(name="consts", bufs=1))

    # ---- input DMAs: weights + first blocks first, alternate queues ----
    wq = consts.tile([D, NX, D], IN_DT, tag="wq")
    wk = consts.tile([D, NX, D], IN_DT, tag="wk")
    wv = consts.tile([D, D], IN_DT, tag="wv")
    nc.sync.dma_start(out=wk[:, 0, :], in_=wk_d[0])
    nc.gpsimd.dma_start(out=wq[:, 0, :], in_=wq_d[0])
    nc.sync.dma_start(out=wk[:, 1, :], in_=wk_d[1])
    nc.gpsimd.dma_start(out=wq[:, 1, :], in_=wq_d[1])
    nc.gpsimd.dma_start(out=wv, in_=wv_d)
    qkv = consts.tile([D, Lkv], IN_DT, tag="qkv")
    qf = consts.tile([D, L], IN_DT, tag="qf")
    qs_list = [(qkv, qkv_d, c, min(512, Lkv - c))
               for c in range(0, Lkv, 512)]
    qf_list = [(qf, qf_d, j * QT, QT) for j in range(NJQ)]
    # interleave kv/q blocks and alternate queues so both proj streams can
    # start early and neither DMA ring serializes the other
    order = []
    for a, b in zip(qs_list, qf_list + [None] * 9):
        order.append(a)
        if b:
            order.append(b)
    for i, (t, d, c, n) in enumerate(order):
        eng = nc.gpsimd if i % 2 == 0 else nc.sync
        eng.dma_start(out=t[:, c:c + n], in_=d[:, c:c + n])

    q_sp = consts.tile([D, NX, L], bf16, tag="q_sp")
    k_sp = consts.tile([D, NX, Lkv], bf16, tag="k_sp")
    v_sb = consts.tile([128, n_kv, H, DH + 1], bf16, tag="v_sb")

    if compact:
        # validity (pre-broadcast per head host-side) -> ones column of v_sb
        # via strided write on GpSimd (keeps ACT/DVE free; DMA would clobber
        # neighbors: 2-byte elems)
        val16 = consts.tile([128, n_kv * H], bf16, tag="val16")
        nc.sync.dma_start(out=val16, in_=val_d)
        dst = bass.AP(tensor=v_sb.tensor, offset=v_sb.offset + DH,
                      ap=[[n_kv * H * (DH + 1), 128], [DH + 1, n_kv * H]])
        nc.vector.tensor_copy(out=dst, in_=val16)
    else:
        val = consts.tile([128, n_kv], f32, tag="val")
        nc.gpsimd.dma_start(out=val, in_=val_d)
        ones8 = consts.tile([128, 8], f32, tag="ones8")
        nc.gpsimd.memset(ones8, 1.0)

    # ---- PSUM: 3 duo ring slots (6 banks) + 2 stream-accumulator banks ----
    ring = ctx.enter_context(tc.tile_pool(name="ring", bufs=3, space="PSUM"))
    accp = ctx.enter_context(tc.tile_pool(name="accp", bufs=2, space="PSUM"))

    def s_tile():
        return ring.tile([128, 2 * QT], f32, tag="s", name="s")

    # ---- HAM warmup + ACT exp-table prime ----
    warm_in = consts.tile([128, 512], bf16, tag="warm_in")
    nc.gpsimd.memset(warm_in, 0.0)
    wps = accp.tile([128, 512], f32, tag="acc", name="acc")
    for i in range(3):
        nc.tensor.matmul(wps[:, 0:512], lhsT=warm_in[:, 0:128],
                         rhs=warm_in, start=True, stop=True)
    p_warm = consts.tile([128, 128], bf16, tag="p_warm")
    nc.scalar.activation(out=p_warm, in_=warm_in[:, 0:128], func=Exp)

    # ---- ACT/DVE time ledger (HW-calibrated: ACT ~n/1.2+210ns,
    # DVE ~n/0.96+135ns per instruction) ----
    eng_t = {"act": 0.0, "dve": 0.0}

    def pick(n):
        c_act = eng_t["act"] + n / 1.2 + 210
        c_dve = eng_t["dve"] + n / 0.96 + 135
        if c_act <= c_dve:
            eng_t["act"] = c_act
            return "act"
        eng_t["dve"] = c_dve
        return "dve"

    def evac(dst_ap, src_ap, n):
        if pick(n) == "act":
            nc.scalar.activation(out=dst_ap, in_=src_ap, func=Copy)
        else:
            nc.vector.tensor_copy(out=dst_ap, in_=src_ap)

    def proj_k(X):
        col = 0
        while col < Lkv:
            n = min(2 * QT, Lkv - col)
            ps = s_tile()
            for off in range(0, n, 512):
                m = min(512, n - off)
                nc.tensor.matmul(ps[:, off:off + m], lhsT=wk[:, X, :],
                                 rhs=qkv[:, col + off:col + off + m],
                                 start=True, stop=True)
            evac(k_sp[:, X, col:col + n], ps[:, 0:n], n)
            col += n

    def proj_q(X):
        col = 0
        while col < L:
            n = min(2 * QT, L - col)
            ps = s_tile()
            for off in range(0, n, 512):
                m = min(512, n - off)
                nc.tensor.matmul(ps[:, off:off + m], lhsT=wq[:, X, :],
                                 rhs=qf[:, col + off:col + off + m],
                                 start=True, stop=True)
            evac(q_sp[:, X, col:col + n], ps[:, 0:n], n)
            col += n

    def proj_v():
        # batch 4 chunks per PSUM bank -> 1 strided evac per 4 chunks
        for c0 in range(0, n_kv, 4):
            nb = min(4, n_kv - c0)
            vp = accp.tile([128, 512], f32, tag="acc", name="acc")
            for ci in range(nb):
                c = c0 + ci
                nc.tensor.matmul(vp[:, ci * D:(ci + 1) * D],
                                 lhsT=qkv[:, c * 128:(c + 1) * 128],
                                 rhs=wv, start=True, stop=True)
            if compact:
                for ci in range(nb):
                    c = c0 + ci
                    nc.vector.tensor_copy(
                        out=v_sb[:, c, :, 0:DH],
                        in_=vp[:, ci * D:(ci + 1) * D].rearrange(
                            "p (h x) -> p h x", x=DH))
                eng_t["dve"] += nb * D / 0.96 + nb * 135
            else:
                for ci in range(nb):
                    c = c0 + ci
                    nc.vector.tensor_scalar_mul(
                        v_sb[:, c, :, 0:DH],
                        vp[:, ci * D:(ci + 1) * D].rearrange(
                            "p (h x) -> p h x", x=DH),
                        val[:, c:c + 1])
                    nc.vector.tensor_scalar_mul(
                        v_sb[:, c, :, DH:DH + 1],
                        ones8.rearrange("p (h x) -> p h x", x=1),
                        val[:, c:c + 1])

    # ---- attention ----
    p_act = ctx.enter_context(tc.tile_pool(name="p_act", bufs=7))
    p_dve = ctx.enter_context(tc.tile_pool(name="p_dve", bufs=6))
    a_pool = ctx.enter_context(tc.tile_pool(name="a_pool", bufs=3))
    m_pool = ctx.enter_context(tc.tile_pool(name="m_pool", bufs=2))
    misc = ctx.enter_context(tc.tile_pool(name="misc", bufs=2))

    accs = {}    # (jq, X) -> PSUM acc tile (until drained)
    a_sbs = {}   # (jq, X) -> SBUF a_sb tile (until epilogue)

    def stream(jq, X):
        """One (jq, X) pair: both head-pair streams advance chunk-by-chunk.
        S duos use disjoint PE row-bands; the 4 AV matmuls per chunk are
        col-banded and all accumulate into ONE psum bank (disjoint
        partition ranges j=0: rows 0:64, j=1: rows 64:128)."""
        n = 2 * QT
        qs = slice(jq * QT, (jq + 1) * QT)
        acc = accp.tile([128, QT], f32, tag="acc", name="acc")
        accs[(jq, X)] = acc

        def s_duo(c, j):
            ck = slice(c * 128, (c + 1) * 128)
            sp = s_tile()
            for gi in range(2):
                g = 2 * j + gi
                nc.tensor.matmul(
                    sp[:, gi * QT:(gi + 1) * QT],
                    lhsT=k_sp[g * 32:(g + 1) * 32, X, ck],
                    rhs=q_sp[g * 32:(g + 1) * 32, X, qs],
                    start=True, stop=True, tile_position=(g * 32, 0))
            return sp

        sps, ps, avq = {}, {}, []

        def emit_av(c):
            st, en = (c == 0), (c == n_kv - 1)
            for j in (0, 1):
                rhs_p = ps.pop((c, j))
                for gi in range(2):
                    g = 2 * j + gi
                    nc.tensor.matmul(
                        acc[g * 32:g * 32 + DH + 1, :],
                        lhsT=v_sb[:, c, 4 * X + g, :],
                        rhs=rhs_p[:, gi * QT:(gi + 1) * QT],
                        start=st, stop=en, tile_position=(0, g * 32))

        for j in (0, 1):
            sps[(0, j)] = s_duo(0, j)
        for c in range(n_kv):
            for j in (0, 1):
                sp = sps.pop((c, j))
                if pick(n) == "act":
                    p = p_act.tile([128, 2 * QT], bf16, tag="p")
                    nc.scalar.activation(out=p, in_=sp, func=Exp)
                    ps[(c, j)] = p
                else:
                    p16 = p_dve.tile([128, 2 * QT], i16, tag="p16")
                    nc.vector.tensor_scalar(out=p16, in0=sp,
                                            scalar1=EXP_A, scalar2=EXP_B,
                                            op0=mult, op1=add)
                    ps[(c, j)] = p16.bitcast(bf16)
            if c + 1 < n_kv:
                for j in (0, 1):
                    sps[(c + 1, j)] = s_duo(c + 1, j)
            avq.append(c)
            if len(avq) > 3:
                emit_av(avq.pop(0))
        while avq:
            emit_av(avq.pop(0))

    def drain(jq, X, split=False):
        acc = accs.pop((jq, X))
        a_sb = a_pool.tile([128, QT], f32, tag=f"a{X}", name="a")
        if split:
            # tail drain: both engines, half each
            h = QT // 2
            nc.scalar.activation(out=a_sb[:, 0:h], in_=acc[:, 0:h], func=Copy)
            nc.vector.tensor_copy(out=a_sb[:, h:QT], in_=acc[:, h:QT])
        else:
            evac(a_sb, acc, QT)
        a_sbs[(jq, X)] = a_sb

    def epilogue(jq, last=False):
        qs = slice(jq * QT, (jq + 1) * QT)
        pk = misc.tile([8, QT], f32, tag="pk")
        for X in range(2):
            a_sb = a_sbs[(jq, X)]
            # denominator rows {16,48,80,112} -> pk[4X:4X+4]
            src = bass.AP(tensor=a_sb.tensor, offset=a_sb.offset + 16 * QT,
                          ap=[[32 * QT, 4], [1, QT]])
            (nc.sync if X == 0 else nc.gpsimd).dma_start(
                out=pk[4 * X:4 * X + 4, :], in_=src)
        rec = misc.tile([8, QT], f32, tag="rec")
        nc.vector.reciprocal_approx_fast(out=rec, in_=pk)
        eng_t["dve"] += QT / 0.96 + 135
        for X in range(2):
            a_sb = a_sbs.pop((jq, X))
            # broadcast 1/denom to band layout rows g*32:(g+1)*32
            rb = m_pool.tile([128, QT], f32, tag="rb", name="rb")
            nc.sync.dma_start(
                out=rb,
                in_=bass.AP(tensor=rec.tensor,
                            offset=rec.offset + 4 * X * QT,
                            ap=[[QT, 4], [0, 32], [1, QT]]))
            mout = m_pool.tile([128, QT], f32, tag=f"m{X}", name="m")
            if last and X == 1:
                # final tile: exps are done, DVE is free - split the multiply
                h = QT // 2
                nc.gpsimd.tensor_mul(out=mout[:, 0:h], in0=a_sb[:, 0:h],
                                     in1=rb[:, 0:h])
                nc.vector.tensor_mul(out=mout[:, h:QT], in0=a_sb[:, h:QT],
                                     in1=rb[:, h:QT])
            else:
                nc.gpsimd.tensor_mul(out=mout, in0=a_sb, in1=rb)
            # scatter head bands g*32+i -> DRAM channel rows (4X+g)*16+i
            # (one DMA per band: DMA APs support only one strided
            # partition dim)
            for g in range(4):
                h = 4 * X + g
                eng = nc.gpsimd if (g + 2 * X) % 2 == 0 else nc.sync
                eng.dma_start(
                    out=out_d[h * DH:(h + 1) * DH, qs],
                    in_=mout[g * 32:g * 32 + DH, :])

    # ---- emission schedule: drains/epilogues one stream late so the
    # exp queues on ACT/DVE never wait on an AV tail ----
    proj_k(0)
    proj_q(0)
    proj_v()
    stream(0, 0)
    proj_k(1)
    stream(1, 0)
    proj_q(1)
    drain(0, 0)
    stream(2, 0)
    drain(1, 0)
    stream(0, 1)
    drain(2, 0)
    stream(3, 0)
    drain(0, 1)
    epilogue(0)
    stream(1, 1)
    drain(3, 0)
    stream(2, 1)
    drain(1, 1)
    epilogue(1)
    stream(3, 1)
    drain(2, 1)
    epilogue(2)
    drain(3, 1, split=True)
    epilogue(3, last=True)


def _build(n_kv: int, compact: bool) -> "bacc.Bacc":
    Lkv = n_kv * 128
    NX = NXP
    nc = bacc.Bacc("TRN2", target_bir_lowering=False, debug=False,
                   enable_asserts=True, num_devices=B)
    qf_d = nc.dram_tensor("q_full", [D, L], IN_DT, kind="ExternalInput").ap()
    qkv_d = nc.dram_tensor("q_kv", [D, Lkv], IN_DT, kind="ExternalInput").ap()
    wq_d = nc.dram_tensor("wq_sp", [NX, D, D], IN_DT, kind="ExternalInput").ap()
    wk_d = nc.dram_tensor("wk_sp", [NX, D, D], IN_DT, kind="ExternalInput").ap()
    wv_d = nc.dram_tensor("wv_t", [D, D], IN_DT, kind="ExternalInput").ap()
    val_dt = bf16 if compact else f32
    val_shape = [128, n_kv * H] if compact else [128, n_kv]
    val_d = nc.dram_tensor("valid", val_shape, val_dt,
                           kind="ExternalInput").ap()
    out_d = nc.dram_tensor("out", [D, L], f32, kind="ExternalOutput").ap()

    with tile.TileContext(nc) as tc, ExitStack() as ctx:
        _body(ctx, tc, qf_d, qkv_d, wq_d, wk_d, wv_d, val_d, out_d, n_kv,
              compact)
    nc.compile()
    return nc


def _prep_weights(w_mem: np.ndarray, w_query: np.ndarray):
    """Spread head weights into 32-row tile groups (rows 16:32 zero) across
    two planes of 4 head-groups, pre-transposed for use as matmul lhsT.
    Q gets the DH^-0.5 scale."""
    wq_sp = np.zeros((NXP, D, D), np.float32)
    wk_sp = np.zeros((NXP, D, D), np.float32)
    scale = np.float32(DH ** -0.5)
    for X in range(NXP):
        for g in range(4):
            h = 4 * X + g
            wq_sp[X][:, 32 * g:32 * g + DH] = (w_query[DH * h:DH * (h + 1), :] * scale).T
            wk_sp[X][:, 32 * g:32 * g + DH] = w_mem[DH * h:DH * (h + 1), :].T
    wv_t = np.ascontiguousarray(w_mem[D:2 * D, :].T)
    return wq_sp, wk_sp, wv_t


COMPACT_KV = True  # drop masked kv positions host-side (exact: they get a
                   # zero validity column -> contribute 0 to num and denom)


def prepare(queries: np.ndarray, mask: np.ndarray, w_mem: np.ndarray,
            w_query: np.ndarray):
    """Build (compiled program, per-core input maps)."""
    import ml_dtypes
    assert queries.shape == (B, D, L) and mask.shape == (B, L)
    maskf = mask.astype(np.float32)
    kept = [np.nonzero(maskf[b] > 0.0)[0] for b in range(B)]
    if COMPACT_KV and all(len(k) > 0 for k in kept):
        n_kv = max(1, -(-max(len(k) for k in kept) // 128))
        compact = True
    else:
        n_kv = L // 128
        kept = None
        compact = False
    Lkv = n_kv * 128

    key = (n_kv, compact)
    nc = _program_cache.get(key)
    if nc is None:
        nc = _program_cache[key] = _build(n_kv, compact)

    wq_sp, wk_sp, wv_t = _prep_weights(
        w_mem.astype(np.float32), w_query.astype(np.float32))

    in_maps = []
    for b in range(B):
        qb = np.ascontiguousarray(queries[b], dtype=np.float32)
        if kept is not None:
            idx = kept[b]
            qkv = np.zeros((D, Lkv), np.float32)
            qkv[:, :len(idx)] = qb[:, idx]
            val = np.zeros(Lkv, np.float32)
            val[:len(idx)] = 1.0
        else:
            qkv = qb
            val = maskf[b]
        valT = np.ascontiguousarray(val.reshape(n_kv, 128).T)
        in_maps.append({
            "q_full": qb,
            "q_kv": np.ascontiguousarray(qkv),
            "wq_sp": wq_sp,
            "wk_sp": wk_sp,
            "wv_t": wv_t,
            "valid": (np.ascontiguousarray(np.repeat(valT, H, axis=1))
                      .astype(ml_dtypes.bfloat16) if compact else valT),
        })
    return nc, in_maps


def kernel(queries: np.ndarray, mask: np.ndarray, w_mem: np.ndarray,
           w_query: np.ndarray) -> np.ndarray:
    nc, in_maps = prepare(queries, mask, w_mem, w_query)
    res = bass_utils.run_bass_kernel_spmd(nc, in_maps, core_ids=list(range(B)))
    return np.stack([res.results[b]["out"] for b in range(B)]).astype(np.float32)


# revision 10
# speedup vs baseline: 1.1003x; 1.1003x over previous
"""Trainium2 Bass kernel for nn_MultiHeadAttnC (QANet-style self-attention).

Reference computation (per batch b):
    memory = w_mem @ queries[b]          # [2D, L]  (pointwise conv)
    query  = w_query @ queries[b]        # [D, L]
    K, V   = heads of memory             # H=8 heads, DH=16
    Q      = heads of query * DH^-0.5
    S      = Q @ K^T  (masked over kv)   # [H, L, L]
    out[b] = softmax(S) @ V  -> recombined to [D, L]

Strategy (v5):
  - Data parallel: batch b -> NeuronCore b. Weights replicated. No collectives.
  - The exp (27M/core, PSUM-resident so only ACT+DVE can touch it) is the
    roofline: ~117us at the two engines' combined ~1.8 cols/ns. Everything
    else is arranged so ACT/DVE do almost nothing but exp:
      - ONE PSUM accumulator bank per (jq, X) stream: both j-streams'
        AV quads write disjoint partition ranges of the same bank
        (emit 4 col-tiled matmuls), so a stream drains with a single
        [128, 512] copy. accp bufs=2 double-buffers across streams.
      - Drains and epilogues are EMITTED one stream late, so on each
        engine's in-order queue they sit behind ~13 chunks of exps and
        never stall waiting for the AV tail.
      - Epilogue: per (jq, X) one strided pk DMA (denominator rows), one
        broadcast rb DMA (band layout), final multiply on GpSimd (SBUF
        only), one scatter DMA straight to DRAM (head-major remap in the
        DMA AP). No xt staging, no engine work besides 1 reciprocal.
      - proj-V evacuation batches 4 chunks per PSUM bank -> 1 strided
        copy per 4 chunks.
  - K-major attention, heads split 4+4 over two weight planes; S^T duo
    tiles are [128 kv, 2x512 q] (2 banks), quad row-banded matmuls merge
    on the PE (~216ns/group measured). Masked kv compacted host-side.
  - exp split between ACT (native Exp) and DVE (single-op Schraudolph:
    i16 = round(x*A + B) == bf16 bits of exp(x)) by a greedy time ledger
    with HW-calibrated costs.
"""

import numpy as np
from contextlib import ExitStack

import concourse.bass as bass
import concourse.tile as tile
from concourse import bacc, mybir
from concourse import bass_utils

B, D, L, H, DH = 8, 128, 2048, 8, 16
f32 = mybir.dt.float32
bf16 = mybir.dt.bfloat16
i16 = mybir.dt.int16
f32r = mybir.dt.float32r
IN_DT = f32r
QT = 512             # q columns per stream tile
NJQ = L // QT        # 4
NXP = 2              # weight spread planes (4 head-groups each)

# Schraudolph exp constants for round-to-nearest f32->i16 conversion:
# bf16_bits(exp(x)) ~= round(x * 2^7*log2(e) + (127*2^7 - 5.6))
EXP_A = 184.6649652337873
EXP_B = 16250.4

_program_cache: dict = {}


def _body(ctx, tc, qf_d, qkv_d, wq_d, wk_d, wv_d, val_d, out_d, n_kv, compact):
    nc = tc.nc
    Lkv = n_kv * 128
    Exp = mybir.ActivationFunctionType.Exp
    Copy = mybir.ActivationFunctionType.Copy
    mult, add = mybir.AluOpType.mult, mybir.AluOpType.add
    NX = NXP

    consts = ctx.enter_context(tc.tile_pool(name="consts", bufs=1))

    # ---- input DMAs: weights + first blocks first. SP and ACT are the two
    # HW-DGE engines (gpsimd DMA is slow software DGE) - at startup ACT is
    # idle, so spread the input over all three descriptor streams ----
    wq = consts.tile([D, NX, D], IN_DT, tag="wq")
    wk = consts.tile([D, NX, D], IN_DT, tag="wk")
    wv = consts.tile([D, D], IN_DT, tag="wv")
    nc.sync.dma_start(out=wk[:, 0, :], in_=wk_d[0])
    nc.scalar.dma_start(out=wq[:, 0, :], in_=wq_d[0])
    nc.sync.dma_start(out=wk[:, 1, :], in_=wk_d[1])
    nc.scalar.dma_start(out=wq[:, 1, :], in_=wq_d[1])
    nc.gpsimd.dma_start(out=wv, in_=wv_d)
    qkv = consts.tile([D, Lkv], IN_DT, tag="qkv")
    qf = consts.tile([D, L], IN_DT, tag="qf")
    qs_list = [(qkv, qkv_d, c, min(512, Lkv - c))
               for c in range(0, Lkv, 512)]
    qf_list = [(qf, qf_d, j * QT, QT) for j in range(NJQ)]
    # interleave kv/q blocks so both proj streams can start early
    order = []
    for a, b in zip(qs_list, qf_list + [None] * 9):
        order.append(a)
        if b:
            order.append(b)
    engs = [nc.sync, nc.scalar, nc.gpsimd]
    for i, (t, d, c, n) in enumerate(order):
        engs[i % 3].dma_start(out=t[:, c:c + n], in_=d[:, c:c + n])

    q_sp = consts.tile([D, NX, L], bf16, tag="q_sp")
    k_sp = consts.tile([D, NX, Lkv], bf16, tag="k_sp")
    v_sb = consts.tile([128, n_kv, H, DH + 1], bf16, tag="v_sb")

    if compact:
        # validity (pre-broadcast per head host-side) -> ones column of v_sb
        # via strided write on GpSimd (keeps ACT/DVE free; DMA would clobber
        # neighbors: 2-byte elems)
        val16 = consts.tile([128, n_kv * H], bf16, tag="val16")
        nc.sync.dma_start(out=val16, in_=val_d)
        dst = bass.AP(tensor=v_sb.tensor, offset=v_sb.offset + DH,
                      ap=[[n_kv * H * (DH + 1), 128], [DH + 1, n_kv * H]])
        nc.vector.tensor_copy(out=dst, in_=val16)
    else:
        val = consts.tile([128, n_kv], f32, tag="val")
        nc.gpsimd.dma_start(out=val, in_=val_d)
        ones8 = consts.tile([128, 8], f32, tag="ones8")
        nc.gpsimd.memset(ones8, 1.0)

    # ---- PSUM: 3 duo ring slots (6 banks) + 2 stream-accumulator banks ----
    ring = ctx.enter_context(tc.tile_pool(name="ring", bufs=3, space="PSUM"))
    accp = ctx.enter_context(tc.tile_pool(name="accp", bufs=2, space="PSUM"))

    def s_tile():
        return ring.tile([128, 2 * QT], f32, tag="s", name="s")

    # ---- HAM warmup + ACT exp-table prime ----
    warm_in = consts.tile([128, 512], bf16, tag="warm_in")
    nc.gpsimd.memset(warm_in, 0.0)
    wps = accp.tile([128, 512], f32, tag="acc", name="acc")
    for i in range(3):
        nc.tensor.matmul(wps[:, 0:512], lhsT=warm_in[:, 0:128],
                         rhs=warm_in, start=True, stop=True)
    p_warm = consts.tile([128, 128], bf16, tag="p_warm")
    nc.scalar.activation(out=p_warm, in_=warm_in[:, 0:128], func=Exp)

    # ---- ACT/DVE time ledger (HW-calibrated: ACT ~n/1.2+210ns,
    # DVE ~n/0.96+135ns per instruction) ----
    eng_t = {"act": 0.0, "dve": 0.0}

    def pick(n):
        c_act = eng_t["act"] + n / 1.2 + 210
        c_dve = eng_t["dve"] + n / 0.96 + 135
        if c_act <= c_dve:
            eng_t["act"] = c_act
            return "act"
        eng_t["dve"] = c_dve
        return "dve"

    def evac(dst_ap, src_ap, n):
        if pick(n) == "act":
            nc.scalar.activation(out=dst_ap, in_=src_ap, func=Copy)
        else:
            nc.vector.tensor_copy(out=dst_ap, in_=src_ap)

    def proj_k(X):
        col = 0
        while col < Lkv:
            n = min(2 * QT, Lkv - col)
            ps = s_tile()
            for off in range(0, n, 512):
                m = min(512, n - off)
                nc.tensor.matmul(ps[:, off:off + m], lhsT=wk[:, X, :],
                                 rhs=qkv[:, col + off:col + off + m],
                                 start=True, stop=True)
            evac(k_sp[:, X, col:col + n], ps[:, 0:n], n)
            col += n

    def proj_q(X):
        col = 0
        while col < L:
            n = min(2 * QT, L - col)
            ps = s_tile()
            for off in range(0, n, 512):
                m = min(512, n - off)
                nc.tensor.matmul(ps[:, off:off + m], lhsT=wq[:, X, :],
                                 rhs=qf[:, col + off:col + off + m],
                                 start=True, stop=True)
            evac(q_sp[:, X, col:col + n], ps[:, 0:n], n)
            col += n

    def proj_v():
        # batch 4 chunks per PSUM bank -> 1 strided evac per 4 chunks
        for c0 in range(0, n_kv, 4):
            nb = min(4, n_kv - c0)
            vp = accp.tile([128, 512], f32, tag="acc", name="acc")
            for ci in range(nb):
                c = c0 + ci
                nc.tensor.matmul(vp[:, ci * D:(ci + 1) * D],
                                 lhsT=qkv[:, c * 128:(c + 1) * 128],
                                 rhs=wv, start=True, stop=True)
            if compact:
                for ci in range(nb):
                    c = c0 + ci
                    nc.vector.tensor_copy(
                        out=v_sb[:, c, :, 0:DH],
                        in_=vp[:, ci * D:(ci + 1) * D].rearrange(
                            "p (h x) -> p h x", x=DH))
                eng_t["dve"] += nb * D / 0.96 + nb * 135
            else:
                for ci in range(nb):
                    c = c0 + ci
                    nc.vector.tensor_scalar_mul(
                        v_sb[:, c, :, 0:DH],
                        vp[:, ci * D:(ci + 1) * D].rearrange(
                            "p (h x) -> p h x", x=DH),
                        val[:, c:c + 1])
                    nc.vector.tensor_scalar_mul(
                        v_sb[:, c, :, DH:DH + 1],
                        ones8.rearrange("p (h x) -> p h x", x=1),
                        val[:, c:c + 1])

    # ---- attention ----
    # P tiles now live up to 1.5 streams (batched AV runs): up to ~26 alive
    p_act = ctx.enter_context(tc.tile_pool(name="p_act", bufs=16))
    p_dve = ctx.enter_context(tc.tile_pool(name="p_dve", bufs=14))
    a_pool = ctx.enter_context(tc.tile_pool(name="a_pool", bufs=4))
    m_pool = ctx.enter_context(tc.tile_pool(name="m_pool", bufs=2))
    misc = ctx.enter_context(tc.tile_pool(name="misc", bufs=2))

    accs = {}    # (jq, X) -> PSUM acc tile (until drained)
    a_sbs = {}   # (jq, X) -> SBUF a_sb tile (until epilogue)
    pend = []    # deferred emission: list of (at_chunk, closure), flushed
                 # inside the NEXT stream's chunk loop

    def stream(jq, X, last=False):
        """One (jq, X) pair: both head-pair streams advance chunk-by-chunk.
        S duos use disjoint PE row-bands. AV matmuls are BATCHED into two
        half-stream runs of col-banded quads (all 4 g-bands -> one psum
        bank, partition-disjoint) so the PE does long uniform runs of one
        matmul configuration instead of thrashing S-quad/AV-quad state
        every chunk. The 2nd-half run + drain + epilogue are deferred into
        the next stream's chunk loop, where the ring keeps ACT/DVE fed."""
        n = 2 * QT
        qs = slice(jq * QT, (jq + 1) * QT)
        acc = accp.tile([128, QT], f32, tag="acc", name="acc")
        accs[(jq, X)] = acc

        def s_duo(c, j):
            ck = slice(c * 128, (c + 1) * 128)
            sp = s_tile()
            for gi in range(2):
                g = 2 * j + gi
                nc.tensor.matmul(
                    sp[:, gi * QT:(gi + 1) * QT],
                    lhsT=k_sp[g * 32:(g + 1) * 32, X, ck],
                    rhs=q_sp[g * 32:(g + 1) * 32, X, qs],
                    start=True, stop=True, tile_position=(g * 32, 0))
            return sp

        sps, ps = {}, {}

        def emit_av(c):
            st, en = (c == 0), (c == n_kv - 1)
            for j in (0, 1):
                rhs_p = ps.pop((c, j))
                for gi in range(2):
                    g = 2 * j + gi
                    nc.tensor.matmul(
                        acc[g * 32:g * 32 + DH + 1, :],
                        lhsT=v_sb[:, c, 4 * X + g, :],
                        rhs=rhs_p[:, gi * QT:(gi + 1) * QT],
                        start=st, stop=en, tile_position=(0, g * 32))

        half = n_kv // 2  # first AV run covers chunks [0, half)

        def run_b():
            for c in range(half, n_kv):
                emit_av(c)

        for j in (0, 1):
            sps[(0, j)] = s_duo(0, j)
        prev = pend[:]
        pend.clear()
        for c in range(n_kv):
            for j in (0, 1):
                sp = sps.pop((c, j))
                if pick(n) == "act":
                    p = p_act.tile([128, 2 * QT], bf16, tag="p")
                    nc.scalar.activation(out=p, in_=sp, func=Exp)
                    ps[(c, j)] = p
                else:
                    p16 = p_dve.tile([128, 2 * QT], i16, tag="p16")
                    nc.vector.tensor_scalar(out=p16, in0=sp,
                                            scalar1=EXP_A, scalar2=EXP_B,
                                            op0=mult, op1=add)
                    ps[(c, j)] = p16.bitcast(bf16)
            if c + 1 < n_kv:
                for j in (0, 1):
                    sps[(c + 1, j)] = s_duo(c + 1, j)
            for at, fn in prev:
                if at == c:
                    fn()
        for at, fn in prev:
            if at >= n_kv:
                fn()
        for c in range(half):
            emit_av(c)
        if last:
            run_b()
        else:
            pend.append((5, run_b))

    def drain(jq, X, split=False):
        acc = accs.pop((jq, X))
        a_sb = a_pool.tile([128, QT], f32, tag=f"a{X}", name="a")
        if split:
            # tail drain: both engines, half each
            h = QT // 2
            nc.scalar.activation(out=a_sb[:, 0:h], in_=acc[:, 0:h], func=Copy)
            nc.vector.tensor_copy(out=a_sb[:, h:QT], in_=acc[:, h:QT])
        else:
            evac(a_sb, acc, QT)
        a_sbs[(jq, X)] = a_sb

    def epilogue(jq, last=False):
        qs = slice(jq * QT, (jq + 1) * QT)
        pk = misc.tile([8, QT], f32, tag="pk")
        for X in range(2):
            a_sb = a_sbs[(jq, X)]
            # denominator rows {16,48,80,112} -> pk[4X:4X+4]
            src = bass.AP(tensor=a_sb.tensor, offset=a_sb.offset + 16 * QT,
                          ap=[[32 * QT, 4], [1, QT]])
            (nc.sync if X == 0 else nc.gpsimd).dma_start(
                out=pk[4 * X:4 * X + 4, :], in_=src)
        rec = misc.tile([8, QT], f32, tag="rec")
        nc.vector.reciprocal_approx_fast(out=rec, in_=pk)
        eng_t["dve"] += QT / 0.96 + 135
        for X in range(2):
            a_sb = a_sbs.pop((jq, X))
            # broadcast 1/denom to band layout rows g*32:(g+1)*32
            rb = m_pool.tile([128, QT], f32, tag="rb", name="rb")
            nc.sync.dma_start(
                out=rb,
                in_=bass.AP(tensor=rec.tensor,
                            offset=rec.offset + 4 * X * QT,
                            ap=[[QT, 4], [0, 32], [1, QT]]))
            mout = m_pool.tile([128, QT], f32, tag=f"m{X}", name="m")
            if last and X == 1:
                # final tile: exps are done, DVE is free - split the multiply
                h = QT // 2
                nc.gpsimd.tensor_mul(out=mout[:, 0:h], in0=a_sb[:, 0:h],
                                     in1=rb[:, 0:h])
                nc.vector.tensor_mul(out=mout[:, h:QT], in0=a_sb[:, h:QT],
                                     in1=rb[:, h:QT])
            else:
                nc.gpsimd.tensor_mul(out=mout, in0=a_sb, in1=rb)
            # scatter head bands g*32+i -> DRAM channel rows (4X+g)*16+i
            # (one DMA per band: DMA APs support only one strided
            # partition dim)
            for g in range(4):
                h = 4 * X + g
                eng = nc.gpsimd if (g + 2 * X) % 2 == 0 else nc.sync
                eng.dma_start(
                    out=out_d[h * DH:(h + 1) * DH, qs],
                    in_=mout[g * 32:g * 32 + DH, :])

    # ---- emission schedule: each stream's 2nd-half AV run (at=5), drain
    # (at=8) and epilogue (at=11) are deferred into the NEXT stream's chunk
    # loop, where the full ring keeps ACT/DVE fed while the PE catches up ----
    proj_k(0)
    proj_q(0)
    proj_v()
    stream(0, 0)
    pend.append((8, lambda: drain(0, 0)))
    proj_k(1)
    stream(1, 0)
    pend.append((8, lambda: drain(1, 0)))
    proj_q(1)
    stream(2, 0)
    pend.append((8, lambda: drain(2, 0)))
    stream(0, 1)
    pend.append((8, lambda: drain(0, 1)))
    pend.append((11, lambda: epilogue(0)))
    stream(3, 0)
    pend.append((8, lambda: drain(3, 0)))
    stream(1, 1)
    pend.append((8, lambda: drain(1, 1)))
    pend.append((11, lambda: epilogue(1)))
    stream(2, 1)
    pend.append((8, lambda: drain(2, 1)))
    pend.append((11, lambda: epilogue(2)))
    stream(3, 1, last=True)
    drain(3, 1, split=True)
    epilogue(3, last=True)


def _build(n_kv: int, compact: bool) -> "bacc.Bacc":
    Lkv = n_kv * 128
    NX = NXP
    nc = bacc.Bacc("TRN2", target_bir_lowering=False, debug=False,
                   enable_asserts=True, num_devices=B)
    qf_d = nc.dram_tensor("q_full", [D, L], IN_DT, kind="ExternalInput").ap()
    qkv_d = nc.dram_tensor("q_kv", [D, Lkv], IN_DT, kind="ExternalInput").ap()
    wq_d = nc.dram_tensor("wq_sp", [NX, D, D], IN_DT, kind="ExternalInput").ap()
    wk_d = nc.dram_tensor("wk_sp", [NX, D, D], IN_DT, kind="ExternalInput").ap()
    wv_d = nc.dram_tensor("wv_t", [D, D], IN_DT, kind="ExternalInput").ap()
    val_dt = bf16 if compact else f32
    val_shape = [128, n_kv * H] if compact else [128, n_kv]
    val_d = nc.dram_tensor("valid", val_shape, val_dt,
                           kind="ExternalInput").ap()
    out_d = nc.dram_tensor("out", [D, L], f32, kind="ExternalOutput").ap()

    with tile.TileContext(nc) as tc, ExitStack() as ctx:
        _body(ctx, tc, qf_d, qkv_d, wq_d, wk_d, wv_d, val_d, out_d, n_kv,
              compact)
    nc.compile()
    return nc


def _prep_weights(w_mem: np.ndarray, w_query: np.ndarray):
    """Spread head weights into 32-row tile groups (rows 16:32 zero) across
    two planes of 4 head-groups, pre-transposed for use as matmul lhsT.
    Q gets the DH^-0.5 scale."""
    wq_sp = np.zeros((NXP, D, D), np.float32)
    wk_sp = np.zeros((NXP, D, D), np.float32)
    scale = np.float32(DH ** -0.5)
    for X in range(NXP):
        for g in range(4):
            h = 4 * X + g
            wq_sp[X][:, 32 * g:32 * g + DH] = (w_query[DH * h:DH * (h + 1), :] * scale).T
            wk_sp[X][:, 32 * g:32 * g + DH] = w_mem[DH * h:DH * (h + 1), :].T
    wv_t = np.ascontiguousarray(w_mem[D:2 * D, :].T)
    return wq_sp, wk_sp, wv_t


COMPACT_KV = True  # drop masked kv positions host-side (exact: they get a
                   # zero validity column -> contribute 0 to num and denom)


def prepare(queries: np.ndarray, mask: np.ndarray, w_mem: np.ndarray,
            w_query: np.ndarray):
    """Build (compiled program, per-core input maps)."""
    import ml_dtypes
    assert queries.shape == (B, D, L) and mask.shape == (B, L)
    maskf = mask.astype(np.float32)
    kept = [np.nonzero(maskf[b] > 0.0)[0] for b in range(B)]
    if COMPACT_KV and all(len(k) > 0 for k in kept):
        n_kv = max(1, -(-max(len(k) for k in kept) // 128))
        compact = True
    else:
        n_kv = L // 128
        kept = None
        compact = False
    Lkv = n_kv * 128

    key = (n_kv, compact)
    nc = _program_cache.get(key)
    if nc is None:
        nc = _program_cache[key] = _build(n_kv, compact)

    wq_sp, wk_sp, wv_t = _prep_weights(
        w_mem.astype(np.float32), w_query.astype(np.float32))

    in_maps = []
    for b in range(B):
        qb = np.ascontiguousarray(queries[b], dtype=np.float32)
        if kept is not None:
            idx = kept[b]
            qkv = np.zeros((D, Lkv), np.float32)
            qkv[:, :len(idx)] = qb[:, idx]
            val = np.zeros(Lkv, np.float32)
            val[:len(idx)] = 1.0
        else:
            qkv = qb
            val = maskf[b]
        valT = np.ascontiguousarray(val.reshape(n_kv, 128).T)
        in_maps.append({
            "q_full": qb,
            "q_kv": np.ascontiguousarray(qkv),
            "wq_sp": wq_sp,
            "wk_sp": wk_sp,
            "wv_t": wv_t,
            "valid": (np.ascontiguousarray(np.repeat(valT, H, axis=1))
                      .astype(ml_dtypes.bfloat16) if compact else valT),
        })
    return nc, in_maps


def kernel(queries: np.ndarray, mask: np.ndarray, w_mem: np.ndarray,
           w_query: np.ndarray) -> np.ndarray:
    nc, in_maps = prepare(queries, mask, w_mem, w_query)
    res = bass_utils.run_bass_kernel_spmd(nc, in_maps, core_ids=list(range(B)))
    return np.stack([res.results[b]["out"] for b in range(B)]).astype(np.float32)
